# revision 2
# baseline (speedup 1.0000x reference)
"""Trainium2 Bass kernel for nn_HcPost:

    out[b,s,n,d] = post[b,s,n] * x[b,s,d] + sum_m comb[b,s,m,n] * residual[b,s,m,d]

Strategy: per token this is a tiny K=5 contraction
    out[n,d] = sum_{m'} Caug[m',n] * Xaug[m',d]
with Xaug = [x; residual_0..3] and Caug = [post; comb_0..3].

We batch G=25 tokens into one TensorE matmul by building a block-diagonal
stationary weight matrix W[(t,m'), (t,n)] = Caug[t,m',n] (K=125, MF=100) on the
host, and streaming Xaug[(t,m'), d] as the moving operand. PSUM results
[(t,n), d] are evacuated to SBUF by VectorE/ScalarE and DMA'd out.

The whole datapath runs in fp16 (host casts inputs, device emits fp16, host
upcasts the result): halves HBM/DMA traffic (the bottleneck for this
memory-regime problem) and runs the PE at 1 cycle/row instead of fp32's 4.
fp16's 10-bit mantissa keeps the end-to-end max-rel error ~1e-3, well under
the 2e-2 gate.

Sharding: tokens (B*S = 16384) split evenly across 8 NeuronCores (data
parallel, no cross-core communication). Tokens are padded to 2050/core so each
core runs 82 uniform groups of 25.
"""

import sys

sys.path.insert(0, "/opt/trn_rl_repo")

import numpy as np

import concourse.bass as bass
import concourse.mybir as mybir
import concourse.tile as tile
from concourse import bacc
from concourse.bass_utils import run_bass_kernel_spmd

B, S, M, N, D = 4, 4096, 4, 4, 2048
TOK = B * S  # 16384 tokens
N_CORES = 8
G = 25  # tokens per PE group (contraction K = 5*G = 125 <= 128)
KDIM = 5 * G  # 125
MF = N * G  # 100 output partitions per group
TPC = 2050  # padded tokens per core (= 82 * 25)
NG = TPC // G  # 82 groups per core
TOKP = TPC * N_CORES  # 16400 padded tokens total
GP = 4  # groups per DMA chunk (batches DMAs to ~4 MB)
DCH = 512  # moving free-dim chunk (one PSUM bank)

_CHUNKS = []
_g = 0
while _g < NG:
    _CHUNKS.append((_g, min(GP, NG - _g)))
    _g += _CHUNKS[-1][1]

# Stashed BassKernelResults of the last kernel() call (for profiling in test
# harnesses via BASS_TRACE=1).
LAST_RESULTS = None
LAST_IN_MAPS = None

BUILD_KWARGS = dict(
    in_eng="gpsimd",
    gp=1,
    abufs=10,
    obufs=9,
    out_spart=100,
    out_delay=6,
    wsplit=8,
    weng="ginter",
    mm_dtype="float16",
)


def _build_program(in_eng="sync", out_eng="sync", in_split=1, out_split=1,
                   out_hpart=0, gp=GP, abufs=2, obufs=2, pbufs=8,
                   out_spart=0, out_delay=4, wsplit=1, weng="sync", mm_dtype="float32",
                   copy_banks=1):
    """Build the SPMD Bass program.

    in_eng/out_eng: comma-separated engine cycle for input/output DMAs —
    elements from {sync, scalar, gpsimd}. Successive chunks rotate through
    the cycle. in_split/out_split: issue each chunk's DMA as this many
    instructions (split along the partition dim). out_hpart: if >0, rows
    [0, out_hpart) of each output chunk go via sync HWDGE and the rest via
    gpsimd SWDGE (overrides out_eng).
    """
    f32 = mybir.dt.float32
    mmdt = getattr(mybir.dt, mm_dtype)
    nc = bacc.Bacc(None, target_bir_lowering=False)
    xa = nc.dram_tensor("xa", [TPC * 5, D], mmdt, kind="ExternalInput")
    wb = nc.dram_tensor("wb", [KDIM, NG * MF], mmdt, kind="ExternalInput")
    y = nc.dram_tensor("y", [TPC * N, D], mmdt, kind="ExternalOutput")

    def engines(spec):
        return [getattr(nc, e) for e in spec.split(",")]

    in_engs = engines(in_eng)
    out_engs = engines(out_eng)

    chunks = []
    g = 0
    while g < NG:
        chunks.append((g, min(gp, NG - g)))
        g += chunks[-1][1]

    # Row r = t*5 + m' of xa is one (token, m') slice; groups are 125 rows.
    xa_v = xa[:].rearrange("(G p) d -> G p d", p=KDIM)
    # Row r = t*4 + n of y; groups are 100 rows.
    y_v = y[:].rearrange("(G p) d -> G p d", p=MF)

    def split_dma(eng, dst, src, nsplit, pdim):
        if nsplit == 1:
            eng.dma_start(dst, src)
            return
        step = (pdim + nsplit - 1) // nsplit
        for s0 in range(0, pdim, step):
            s1 = min(s0 + step, pdim)
            eng.dma_start(dst[s0:s1], src[s0:s1])

    with tile.TileContext(nc) as tc:
        with (
            tc.tile_pool(name="wpool", bufs=1) as wpool,
            tc.tile_pool(name="apool", bufs=abufs) as apool,
            tc.tile_pool(name="opool", bufs=obufs) as opool,
            tc.tile_pool(name="psum", bufs=pbufs, space=bass.MemorySpace.PSUM) as psum,
        ):
            gper = (NG + wsplit - 1) // wsplit
            interleave_w = weng == "ginter"
            wt_tiles = []
            w_eng = nc.gpsimd if interleave_w else getattr(nc, weng)

            def load_w(wi):
                glo = wi * gper
                ghi = min(NG, (wi + 1) * gper)
                wtile = wpool.tile([KDIM, (ghi - glo) * MF], mmdt, tag=f"w{wi}")
                w_eng.dma_start(wtile[:], wb[:, glo * MF : ghi * MF])
                wt_tiles.append(wtile)

            if not interleave_w:
                for wi in range(wsplit):
                    load_w(wi)

            def w_slice(g):
                wi, off = divmod(g, gper)
                return wt_tiles[wi][:, off * MF : (off + 1) * MF]

            k = 0
            pending = []  # delayed SWDGE output DMAs: (dst_ap, src_tile_ap)
            for ci, (gstart, cgp) in enumerate(chunks):
                a = apool.tile([KDIM, cgp, D], mmdt, tag="a")
                split_dma(
                    in_engs[ci % len(in_engs)],
                    a[:],
                    xa_v[gstart : gstart + cgp].rearrange("g p d -> p g d"),
                    in_split,
                    KDIM,
                )
                if interleave_w and ci < wsplit:
                    load_w(ci)
                if out_spart > 0 and len(pending) >= out_delay:
                    dst, src = pending.pop(0)
                    nc.gpsimd.dma_start(dst, src)
                o = opool.tile([MF, cgp, D], mmdt, tag="o")
                for gs in range(cgp):
                    gw = gstart + gs
                    for dcb in range(0, D // DCH, copy_banks):
                        p = psum.tile([MF, copy_banks * DCH], f32)
                        for j in range(copy_banks):
                            dc = dcb + j
                            nc.tensor.matmul(
                                p[:, j * DCH : (j + 1) * DCH],
                                lhsT=w_slice(gw),
                                rhs=a[:, gs, dc * DCH : (dc + 1) * DCH],
                                start=True,
                                stop=True,
                            )
                        dst = o[:, gs, dcb * DCH : (dcb + copy_banks) * DCH]
                        if k % 2 == 0:
                            nc.vector.tensor_copy(dst, p[:])
                        else:
                            nc.scalar.copy(dst, p[:])
                        k += 1
                y_dst = y_v[gstart : gstart + cgp].rearrange("g p d -> p g d")
                if out_spart > 0:
                    hp = MF - out_spart
                    if hp > 0:
                        nc.sync.dma_start(y_dst[:hp], o[:hp])
                    pending.append((y_dst[hp:], o[hp:]))
                elif out_hpart > 0:
                    nc.sync.dma_start(y_dst[:out_hpart], o[:out_hpart])
                    nc.gpsimd.dma_start(y_dst[out_hpart:], o[out_hpart:])
                else:
                    split_dma(
                        out_engs[ci % len(out_engs)],
                        y_dst,
                        o[:],
                        out_split,
                        MF,
                    )
            for dst, src in pending:
                nc.gpsimd.dma_start(dst, src)
    nc.compile()
    return nc


def kernel(x, residual, post, comb):
    global LAST_RESULTS, LAST_IN_MAPS
    x = np.asarray(x, dtype=np.float32)
    residual = np.asarray(residual, dtype=np.float32)
    post = np.asarray(post, dtype=np.float32)
    comb = np.asarray(comb, dtype=np.float32)

    # Host prepack: augmented data rows (token-major) and block-diagonal
    # weights, cast to fp16. Padded tokens have zero weights -> zero output
    # rows.
    xaug = np.zeros((TOKP, 5, D), np.float16)
    xaug[:TOK, 0, :] = x.reshape(TOK, D)
    xaug[:TOK, 1:, :] = residual.reshape(TOK, M, D)

    caug = np.zeros((TOKP, 5, N), np.float32)
    caug[:TOK, 0, :] = post.reshape(TOK, N)
    caug[:TOK, 1:, :] = comb.reshape(TOK, M, N)

    ngt = TOKP // G  # total groups
    wall = np.zeros((ngt, KDIM, MF), np.float16)
    t = np.arange(G)
    rows = np.broadcast_to(
        5 * t[:, None, None] + np.arange(5)[None, :, None], (G, 5, N)
    ).ravel()
    cols = np.broadcast_to(
        N * t[:, None, None] + np.arange(N)[None, None, :], (G, 5, N)
    ).ravel()
    wall[:, rows, cols] = caug.reshape(ngt, G * 5 * N)

    in_maps = []
    for c in range(N_CORES):
        xa_c = np.ascontiguousarray(xaug[c * TPC : (c + 1) * TPC].reshape(TPC * 5, D))
        wb_c = np.ascontiguousarray(
            wall[c * NG : (c + 1) * NG].transpose(1, 0, 2).reshape(KDIM, NG * MF)
        )
        in_maps.append({"xa": xa_c, "wb": wb_c})

    LAST_IN_MAPS = in_maps
    nc = _build_program(**BUILD_KWARGS)
    res = run_bass_kernel_spmd(nc, in_maps, list(range(N_CORES)))
    LAST_RESULTS = res

    y = np.concatenate(
        [res.results[c]["y"].reshape(TPC, N, D) for c in range(N_CORES)], axis=0
    )[:TOK]
    return np.ascontiguousarray(y.reshape(B, S, N, D).astype(np.float32))
